# revision 21
# baseline (speedup 1.0000x reference)
"""Embedding lookup + small linear projection on 8 Trainium2 NeuronCores.

Computation (full problem):
    rows = user_repost_matrix[input.reshape(-1)]      # [12800, 2000] f32
    out  = rows @ W.T + b                             # [12800, 8]
    out.reshape(64, 200, 8)

Distribution: data-parallel over the 12800 tokens (1600 per core), table
replicated in every core's DRAM (no collectives). The table is staged in
fp16 (rows padded to 2048 elems = 4096B), halving HBM gather traffic; the
dot products are computed in fp16 with fp32 PSUM accumulation (~5e-4 max
rel err, well inside the 2e-2 gate).

Per-core device kernel:
  1. gpsimd.dma_gather(transpose=True) pulls up to 256 table rows per call
     and deposits them TRANSPOSED in SBUF as [128, 16, ntok] fp16 --
     feature f = k*128 + p lands on partition p, chunk k. This removes the
     PE transpose + PSUM round-trip of the previous design entirely.
  2. Per 16 feature-chunks: one fp16 matmul psum[8, ntok] += W_k^T @ rows_k
     (W chunk [128, 8] stationary, gathered tokens moving).
  3. DVE adds bias (per-partition scalar) while copying PSUM -> SBUF f32,
     DMA result slice to DRAM out[8, TOT]; host transposes/unpermutes.

dma_gather indices are int16 (< 32768), so the 100000-row table is split
into 4 base-offset groups of 25000 rows. Tokens are grouped by row-group
on the host, balanced across cores so every core has identical per-group
counts (global pad to a multiple of 8 with dummy index-0 tokens), and each
group is gathered from its own table base AP. Trailing -1 indices pad each
group to a 128-multiple; the gather ucode only transfers up to
roundup(valid, 16) rows, and garbage columns only pollute their own output
column (matmul columns are independent), which the host drops.
"""

import sys

if "/opt/trn_rl_repo" not in sys.path:
    sys.path.insert(0, "/opt/trn_rl_repo")

import numpy as np

import concourse.bass as bass
import concourse.tile as tile
from concourse import bacc, library_config, mybir
from concourse.bass_utils import run_bass_kernel_spmd
from concourse.masks import make_identity

NTOKEN = 100000
D = 2000
DPAD = 2048                      # fp16 row padded to 4096 bytes
J = 8
B, L = 64, 200
N_CORES = 8
TOK = B * L                      # 12800
PER_CORE = TOK // N_CORES        # 1600
NGROUPS = 4
GR = 25000                       # table rows per index group (fits int16)
KCH = DPAD // 128                # 16 feature chunks of 128
SUB = 256                        # tokens per gather / matmul subtile

F32 = mybir.dt.float32
FP16 = mybir.dt.float16
I16 = mybir.dt.int16
I32 = mybir.dt.int32

_cached = {}


def _roundup(x, m):
    return (x + m - 1) // m * m


def _subtiles(n_gs):
    """Static subtile schedule: (group, global col off, size, valid, indirect).

    The first subtile of groups 0 and 1 is serviced by int32
    indirect_dma_start + PE transpose instead of the Q7 dma_gather ucode:
    those DMAs need no library, so they run during the ~7us mlp-library
    load that otherwise leaves the DMA engines idle.
    """
    subs = []
    off = 0
    for g in range(NGROUPS):
        n = n_gs[g]
        cap = _roundup(max(n, 1), 128)
        start = 0
        while start < cap:
            sz = min(SUB, cap - start)
            valid = min(n, start + sz) - start
            subs.append((g, off + start, sz, valid, g < 2 and start == 0))
            start += sz
        off += cap
    return subs, off


def _build(n_gs):
    """Build + compile the SPMD Bass module for per-core group sizes n_gs."""
    subs, tot = _subtiles(n_gs)
    nc = bacc.Bacc(
        "TRN2", target_bir_lowering=False, debug=False, num_devices=N_CORES
    )
    nind = sum(sz // 128 for g, coff, sz, valid, ind in subs if ind)
    table = nc.dram_tensor("table", [NTOKEN, DPAD], FP16, kind="ExternalInput").ap()
    # [128, n/16]: token i of a gather window at [i % 16, i // 16], with the
    # 16-partition block replicated for each of the 8 Q7 cores.
    idxs = nc.dram_tensor("idxs", [128, tot // 16], I16, kind="ExternalInput").ap()
    # int32 global row ids for the indirect-prefetched 128-token blocks
    idx32 = nc.dram_tensor("idx32", [128, nind], I32, kind="ExternalInput").ap()
    wmat = nc.dram_tensor("w", [128, KCH * J], FP16, kind="ExternalInput").ap()
    bias = nc.dram_tensor("bias", [J, 1], F32, kind="ExternalInput").ap()
    out = nc.dram_tensor("out", [J, tot], F32, kind="ExternalOutput").ap()

    with tile.TileContext(nc) as tc:
        with (
            tc.tile_pool(name="const", bufs=1) as cpool,
            tc.tile_pool(name="rows", bufs=2) as rpool,
            tc.tile_pool(name="tp", bufs=2, space="PSUM") as tppool,
            tc.tile_pool(name="rt", bufs=4) as rtpool,
            tc.tile_pool(name="gath", bufs=6) as gpool,
            tc.tile_pool(name="acc", bufs=6, space="PSUM") as ppool,
        ):
            idx32_sb = cpool.tile([128, nind], I32)
            nc.sync.dma_start(idx32_sb[:], idx32[:])
            idx_sb = cpool.tile([128, tot // 16], I16)
            nc.sync.dma_start(idx_sb[:], idxs[:])
            w_sb = cpool.tile([128, KCH * J], FP16)
            nc.sync.dma_start(w_sb[:], wmat[:])
            bias_sb = cpool.tile([J, 1], F32)
            nc.sync.dma_start(bias_sb[:], bias[:])
            ident = cpool.tile([128, 128], FP16)
            make_identity(nc, ident[:])
            out_sb = cpool.tile([J, tot], F32)

            # Indirect-prefetched subtiles: int32 row gather (no library)
            # + PE fp16 transpose + fp16 matmul. Emitted on Pool BEFORE the
            # library load so their DMAs run while the mlp ucode loads.
            icol = 0
            for g, coff, sz, valid, ind in subs:
                if not ind:
                    continue
                for t in range(sz // 128):
                    r = rpool.tile([128, DPAD], FP16)
                    nc.gpsimd.indirect_dma_start(
                        out=r[:],
                        out_offset=None,
                        in_=table[:],
                        in_offset=bass.IndirectOffsetOnAxis(
                            ap=idx32_sb[:, icol : icol + 1], axis=0
                        ),
                    )
                    icol += 1
                    ps = ppool.tile([J, 128], F32, space="PSUM")
                    for k in range(KCH):
                        tp = tppool.tile([128, 128], FP16, space="PSUM")
                        nc.tensor.transpose(
                            out=tp[:],
                            in_=r[:, k * 128 : (k + 1) * 128],
                            identity=ident[:],
                        )
                        rt = rtpool.tile([128, 128], FP16)
                        if k % 2 == 0:
                            nc.scalar.copy(rt[:], tp[:])
                        else:
                            nc.vector.tensor_copy(rt[:], tp[:])
                        nc.tensor.matmul(
                            out=ps[:],
                            lhsT=w_sb[:, k * J : (k + 1) * J],
                            rhs=rt[:],
                            start=(k == 0),
                            stop=(k == KCH - 1),
                            skip_group_check=True,
                        )
                    v = min(valid - t * 128, 128)
                    if v > 0:
                        nc.vector.tensor_scalar_add(
                            out_sb[:, coff + t * 128 : coff + t * 128 + v],
                            ps[:, :v],
                            bias_sb[:, 0:1],
                        )

            # Q7 ucode load for dma_gather; ~7us, overlaps the indirect
            # transfers above.
            nc.gpsimd.load_library(library_config.mlp)

            for g, coff, sz, valid, ind in subs:
                if ind:
                    continue
                gt = gpool.tile([128, KCH, sz], FP16)
                nc.gpsimd.dma_gather(
                    gt[:],
                    table[g * GR : (g + 1) * GR, :],
                    idx_sb[:, coff // 16 : (coff + sz) // 16],
                    sz,
                    valid,
                    DPAD,
                    transpose=True,
                )
                # Only stream the columns that hold real tokens (the gather
                # transfers roundup(valid, 16) rows); keeps PE pace under the
                # ~2.9us/subtile DMA pace and shrinks the tail subtile.
                vr = _roundup(valid, 16)
                ps = ppool.tile([J, sz], F32, space="PSUM")
                for k in range(KCH):
                    nc.tensor.matmul(
                        out=ps[:, :vr],
                        lhsT=w_sb[:, k * J : (k + 1) * J],
                        rhs=gt[:, k, :vr],
                        start=(k == 0),
                        stop=(k == KCH - 1),
                    )
                nc.vector.tensor_scalar_add(
                    out_sb[:, coff : coff + valid],
                    ps[:, :valid],
                    bias_sb[:, 0:1],
                )
            nc.sync.dma_start(out[:], out_sb[:])

    nc.compile()
    return nc


def _get_nc(n_gs):
    key = tuple(n_gs)
    if key not in _cached:
        _cached[key] = _build(key)
    return _cached[key]


def _prep_in_maps(input, user_repost_matrix, W, b):
    idx_full = np.asarray(input).reshape(-1).astype(np.int64)
    assert idx_full.shape[0] == TOK

    # Partition tokens by table row-group, balanced across cores.
    grp = (idx_full // GR).astype(np.int64)
    # core_tok[c][g] -> (local_idx int16 array, orig_pos int64 array)
    core_tok = [[None] * NGROUPS for _ in range(N_CORES)]
    n_gs = []
    for g in range(NGROUPS):
        pos = np.nonzero(grp == g)[0]
        # pad globally to a multiple of N_CORES with dummy tokens (row 0 of
        # this group, orig position -1); keep at least one real slot per
        # core so no gather ends up with zero valid indices
        npad = _roundup(max(len(pos), 1), N_CORES) - len(pos)
        loc = (idx_full[pos] - g * GR).astype(np.int16)
        if npad:
            loc = np.concatenate([loc, np.zeros(npad, np.int16)])
            pos = np.concatenate([pos, np.full(npad, -1, np.int64)])
        n_gs.append(len(pos) // N_CORES)
        for c in range(N_CORES):
            core_tok[c][g] = (loc[c::N_CORES], pos[c::N_CORES])
    n_gs = tuple(n_gs)
    subs, tot = _subtiles(n_gs)

    table16 = np.zeros((NTOKEN, DPAD), dtype=np.float16)
    table16[:, :D] = np.asarray(user_repost_matrix, dtype=np.float32)

    # w_sb[p, k*8 + j] = W.T[k*128 + p, j]
    wt = np.zeros((DPAD, J), dtype=np.float16)
    wt[:D] = np.asarray(W, dtype=np.float32).T
    w_sb = np.ascontiguousarray(
        wt.reshape(KCH, 128, J).transpose(1, 0, 2).reshape(128, KCH * J)
    )
    bias_arr = np.ascontiguousarray(
        np.asarray(b, dtype=np.float32).reshape(J, 1)
    )

    in_maps = []
    pos_flat_all = []
    for c in range(N_CORES):
        idx_flat = np.full(tot, -1, np.int16)
        pos_flat = np.full(tot, -1, np.int64)
        off = 0
        for g in range(NGROUPS):
            loc, pos = core_tok[c][g]
            n = n_gs[g]
            idx_flat[off : off + n] = loc
            pos_flat[off : off + n] = pos
            off += _roundup(max(n, 1), 128)
        assert off == tot
        # idx_dram[r, col]: flat token p lives at [p % 16, p // 16]; the
        # 16-row block is tiled 8x down the partition dim (one copy per
        # Q7 core).
        idx_arr = np.ascontiguousarray(
            np.tile(idx_flat.reshape(tot // 16, 16).T, (N_CORES, 1))
        )
        # int32 global row ids for the indirect-prefetched 128-blocks
        i32cols = []
        for g, coff, sz, valid, ind in subs:
            if not ind:
                continue
            for t in range(sz // 128):
                blk = idx_flat[coff + t * 128 : coff + (t + 1) * 128]
                i32cols.append(
                    np.maximum(blk.astype(np.int32), 0) + g * GR
                )
        idx32_arr = np.ascontiguousarray(np.stack(i32cols, axis=1))
        pos_flat_all.append(pos_flat)
        in_maps.append(
            {
                "table": table16,
                "idxs": idx_arr,
                "idx32": idx32_arr,
                "w": w_sb,
                "bias": bias_arr,
            }
        )
    return in_maps, n_gs, pos_flat_all


def _run(in_maps, n_gs, trace=False, **kw):
    nc = _get_nc(n_gs)
    return run_bass_kernel_spmd(
        nc, in_maps, list(range(N_CORES)), trace=trace, **kw
    )


def _unshard(results, pos_flat_all):
    full = np.empty((TOK, J), dtype=np.float32)
    for c in range(N_CORES):
        res = results[c]["out"]                     # [8, tot] f32
        pos = pos_flat_all[c]
        valid = pos >= 0
        full[pos[valid]] = res.T[valid]
    return full.reshape(B, L, J)


def kernel(input, user_repost_matrix, W, b):
    in_maps, n_gs, pos_all = _prep_in_maps(input, user_repost_matrix, W, b)
    res = _run(in_maps, n_gs)
    return _unshard(res.results, pos_all)
